# revision 1
# baseline (speedup 1.0000x reference)
"""Trainium2 Bass kernel for DynamicHyperedgeWeightLearner.

Strategy (8 NeuronCores, SPMD single NEFF):
  - Hyperedges (M=4096) are degree-sorted and dealt round-robin into
    8 cores x 4 groups x 128 edges, so every core sees identical group
    shapes (required: one NEFF for all cores).
  - Per core: PE computes transposed stats sum(x)^T / sum(x^2)^T by
    streaming the core's H^T slab (bf16, exact for 0/1) as the moving
    matmul operand against stationary [x | x^2] bf16 chunks.  The
    output lands feature-major, which is exactly the layout the MLP
    needs, so no PE transposes are required.  1/clip(deg,1) is
    precomputed on host and applied as a broadcast row.
  - delta (max-min over members) uses the sparse structure: bf16
    transpose-mode dma_gather lands each member row's 128 dims on the
    128 partitions with members along the free axis (edge-major), then
    DVE reduce_max/min over each edge's contiguous member run gives
    delta^T directly.  Gathers are chunked to <=1024 indices (SWDGE
    descriptor-ring capacity).
  - The tiny MLP runs feature-major on PE; t_embed is folded into an
    effective b1 on the host since it is constant across hyperedges.

bf16 note: H is exactly representable; x / x^2 rounding changes the
final sigmoid outputs by ~1e-4 relative (measured), far below fp32
matmul noise thresholds that matter here.
"""

import numpy as np
import ml_dtypes

N, M, D = 8192, 4096, 128
P = 128
C = 8                 # cores
GROUPS = 4            # groups of 128 edges per core
MC = GROUPS * P       # 512 edges per core
KT = N // P           # 64 k-tiles
KB = 8                # k-tiles per DMA block
T_DIM, MLP_H = 32, 64
RING = 1024           # SWDGE descriptor-ring capacity (scratch/16)

last_run_info = {}


def _chunks(kg):
    """Per-group gather chunking: list of (e0, ec, L) per group, where a
    chunk covers edges [e0, e0+ec) with L = roundup(ec*K, 128) indices."""
    out = []
    for K in kg:
        E = max(1, RING // K)
        ch = []
        for e0 in range(0, P, E):
            ec = min(E, P - e0)
            L = -(-(ec * K) // 128) * 128
            ch.append((e0, ec, L))
        out.append(ch)
    return out


def _prep(node_embeddings, incidence_matrix, time_step,
          W_t, b_t, W1, b1, W2, b2, W3, b3):
    """Host-side preprocessing -> per-core input maps + assembly info."""
    bf16 = ml_dtypes.bfloat16
    x = np.ascontiguousarray(node_embeddings, dtype=np.float32)
    H = np.ascontiguousarray(incidence_matrix, dtype=np.float32)

    nodes, edges = np.nonzero(H)
    order = np.argsort(edges, kind="stable")
    n_sorted = nodes[order].astype(np.int32)
    deg = np.bincount(edges, minlength=M).astype(np.int64)
    offs = np.zeros(M + 1, np.int64)
    np.cumsum(deg, out=offs[1:])

    rank = np.argsort(-deg, kind="stable")      # edge ids by degree desc
    kg = [int(max(1, deg[rank[g * 1024:(g + 1) * 1024]].max()))
          for g in range(GROUPS)]
    chunks = _chunks(kg)

    slot = np.arange(MC)
    eids = []                                   # per-core edge ids, (512,)
    idx_cores = []                              # per-core idx sbuf arrays
    for c in range(C):
        e = rank[(slot // P) * 1024 + c * P + (slot % P)]
        eids.append(e)
        parts = []
        for g in range(GROUPS):
            K = kg[g]
            Jm = np.full((K, P), N, dtype=np.int16)      # default: zero row
            for p in range(P):
                eid = e[g * P + p]
                d = int(deg[eid])
                if d > 0:
                    mem = n_sorted[offs[eid]:offs[eid] + d]
                    Jm[:d, p] = mem[:K]
                    if d < K:
                        Jm[d:, p] = mem[-1]
            lin = Jm.reshape(-1)
            parts.append(np.tile(lin.reshape(-1, 16).T, (8, 1)))
        idx_cores.append(np.ascontiguousarray(np.concatenate(parts, axis=1),
                                              dtype=np.int16))

    # fp32 scalar chain identical to the reference
    t = np.float32(np.asarray(time_step, dtype=np.float32).reshape(()))
    t_embed = np.maximum(
        (t * np.asarray(W_t, np.float32)[:, 0] + np.asarray(b_t, np.float32)),
        np.float32(0.0)).astype(np.float32)
    W1 = np.asarray(W1, np.float32)
    b1_eff = (np.asarray(b1, np.float32)
              + W1[:, 3 * D:] @ t_embed).astype(np.float32)
    w1T = np.ascontiguousarray(W1[:, :3 * D].T)                   # (384, 64)
    w2T = np.ascontiguousarray(np.asarray(W2, np.float32).T)      # (64, 32)
    w3T = np.ascontiguousarray(np.asarray(W3, np.float32).T)      # (32, 1)
    b2 = np.asarray(b2, np.float32).reshape(32, 1).copy()
    b3 = np.asarray(b3, np.float32).reshape(1, 1).copy()

    xc = np.concatenate([x, x * x], axis=1)                       # (N, 256)
    xc_dev = np.ascontiguousarray(
        xc.reshape(KT, P, 2 * D).transpose(1, 0, 2)).astype(bf16)
    xg = np.vstack([x, np.zeros((1, D), np.float32)]).astype(bf16)
    w1T_dev = np.ascontiguousarray(
        w1T.reshape(3, P, MLP_H).transpose(1, 0, 2))              # (128, 3, 64)
    b1_dev = b1_eff.reshape(MLP_H, 1).copy()

    in_maps = []
    for c in range(C):
        Hc = H[:, eids[c]]                                        # (8192, 512)
        h_dev = np.ascontiguousarray(
            Hc.reshape(KT, P, MC).transpose(1, 0, 2)).astype(bf16)
        rd = (1.0 / np.clip(deg[eids[c]], 1, None)).astype(np.float32)
        in_maps.append({
            "hs": h_dev, "xc": xc_dev, "xg": xg, "idx": idx_cores[c],
            "rdeg": rd.reshape(1, MC).copy(), "w1T": w1T_dev, "b1": b1_dev,
            "w2T": w2T, "b2": b2, "w3T": w3T, "b3": b3,
        })
    return in_maps, eids, kg


def _build(kg, loops=1):
    """Build the SPMD Bass program (one NEFF, all 8 cores)."""
    import concourse.mybir as mybir
    import concourse.tile as tile
    from concourse import bacc

    from concourse.masks import make_identity
    f32 = mybir.dt.float32
    bf = mybir.dt.bfloat16
    iw = 8 * sum(kg)

    nc = bacc.Bacc("TRN2")
    hs_d = nc.dram_tensor("hs", [P, KT, MC], bf, kind="ExternalInput")
    xc_d = nc.dram_tensor("xc", [P, KT, 2 * D], bf, kind="ExternalInput")
    xg_d = nc.dram_tensor("xg", [N + 1, D], bf, kind="ExternalInput")
    idx_d = nc.dram_tensor("idx", [P, iw], mybir.dt.int16, kind="ExternalInput")
    rdeg_d = nc.dram_tensor("rdeg", [1, MC], f32, kind="ExternalInput")
    w1T_d = nc.dram_tensor("w1T", [P, 3, MLP_H], f32, kind="ExternalInput")
    b1_d = nc.dram_tensor("b1", [MLP_H, 1], f32, kind="ExternalInput")
    w2T_d = nc.dram_tensor("w2T", [MLP_H, 32], f32, kind="ExternalInput")
    b2_d = nc.dram_tensor("b2", [32, 1], f32, kind="ExternalInput")
    w3T_d = nc.dram_tensor("w3T", [32, 1], f32, kind="ExternalInput")
    b3_d = nc.dram_tensor("b3", [1, 1], f32, kind="ExternalInput")
    out_d = nc.dram_tensor("out", [1, MC], f32, kind="ExternalOutput")

    with tile.TileContext(nc) as tc:
        with (
            tc.tile_pool(name="singles", bufs=1) as singles,
            tc.tile_pool(name="hstream", bufs=3) as hstream,
            tc.tile_pool(name="gpool", bufs=1) as gpool,
            tc.tile_pool(name="stage_p", bufs=4) as stage_p,
            tc.tile_pool(name="mlp", bufs=2) as mlp,
            tc.tile_pool(name="ps_stats", bufs=1, space="PSUM") as ps_stats,
            tc.tile_pool(name="ps_tr", bufs=2, space="PSUM") as ps_tr,
            tc.tile_pool(name="ps_mlp", bufs=1, space="PSUM") as ps_mlp,
        ):
            # ---- resident loads ----
            xc_sb = singles.tile([P, KT, 2 * D], bf)
            nc.sync.dma_start(xc_sb, xc_d[:, :, :])
            idx_sb = singles.tile([P, iw], mybir.dt.int16)
            nc.sync.dma_start(idx_sb, idx_d[:, :])
            rdeg_bc = singles.tile([P, MC], f32)
            nc.gpsimd.dma_start(rdeg_bc, rdeg_d[0:1, :].to_broadcast((P, MC)))
            w1T_sb = singles.tile([P, 3, MLP_H], f32)
            nc.sync.dma_start(w1T_sb, w1T_d[:, :, :])
            b1_sb = singles.tile([MLP_H, 1], f32)
            nc.sync.dma_start(b1_sb, b1_d[:, :])
            w2T_sb = singles.tile([MLP_H, 32], f32)
            nc.sync.dma_start(w2T_sb, w2T_d[:, :])
            b2_sb = singles.tile([32, 1], f32)
            nc.sync.dma_start(b2_sb, b2_d[:, :])
            w3T_sb = singles.tile([32, 1], f32)
            nc.sync.dma_start(w3T_sb, w3T_d[:, :])
            b3_sb = singles.tile([1, 1], f32)
            nc.sync.dma_start(b3_sb, b3_d[:, :])

            ident = singles.tile([P, P], f32)
            make_identity(nc, ident)

            for _rep in range(loops):
                hT = [mlp.tile([P, MC], f32, tag=f"hT{b}", name=f"hT{b}")
                      for b in range(3)]

                # ---- delta: fp32 gathers (edge-major partitions) ----
                # SWDGE ring holds 1024 descriptors -> chunk members at 8.
                CH = 8
                off = 0
                gths = []
                for g in range(GROUPS):
                    K = kg[g]
                    gth = gpool.tile([P, K, D], bf, tag=f"gth{g}",
                                     name=f"gth{g}")
                    for j0 in range(0, K, CH):
                        ch = min(CH, K - j0)
                        nc.gpsimd.dma_gather(
                            gth[:, j0:j0 + ch, :], xg_d[:, :],
                            idx_sb[:, off + 8 * j0:off + 8 * (j0 + ch)],
                            ch * P, ch * P, D,
                        )
                    gths.append(gth)
                    off += 8 * K
                for g in range(GROUPS):
                    gv = gths[g].rearrange("p j d -> p d j")
                    rmax = stage_p.tile([P, D], f32, tag="rmax", name="rmax")
                    rmin = stage_p.tile([P, D], f32, tag="rmin", name="rmin")
                    nc.vector.tensor_reduce(rmax, gv, axis=mybir.AxisListType.X,
                                            op=mybir.AluOpType.max)
                    nc.vector.tensor_reduce(rmin, gv, axis=mybir.AxisListType.X,
                                            op=mybir.AluOpType.min)
                    dl = stage_p.tile([P, D], f32, tag="dl", name="dl")
                    nc.vector.tensor_tensor(dl, rmax, rmin,
                                            mybir.AluOpType.subtract)
                    tps = ps_tr.tile([P, P], f32, tag="tr", name="tr")
                    nc.tensor.transpose(tps, dl, ident)
                    nc.any.tensor_copy(out=hT[2][:, g * P:(g + 1) * P], in_=tps)

                # ---- stats matmuls (transposed): psum = xc_k^T @ H_k ----
                psx = ps_stats.tile([P, MC], f32, tag="psx", name="psx")
                psq = ps_stats.tile([P, MC], f32, tag="psq", name="psq")
                for kb in range(KT // KB):
                    ht = hstream.tile([P, KB, MC], bf)
                    nc.sync.dma_start(ht, hs_d[:, kb * KB:(kb + 1) * KB, :])
                    for sk in range(KB):
                        k = kb * KB + sk
                        nc.tensor.matmul(psx, xc_sb[:, k, 0:D], ht[:, sk, :],
                                         start=(k == 0), stop=(k == KT - 1))
                        nc.tensor.matmul(psq, xc_sb[:, k, D:2 * D], ht[:, sk, :],
                                         start=(k == 0), stop=(k == KT - 1))

                # ---- mu^T / sigma^T ----
                nc.vector.tensor_tensor(hT[0], psx, rdeg_bc,
                                        mybir.AluOpType.mult)
                qT = stage_p.tile([P, MC], f32, tag="qT", name="qT")
                nc.vector.tensor_tensor(qT, psq, rdeg_bc, mybir.AluOpType.mult)
                var = stage_p.tile([P, MC], f32, tag="var", name="var")
                nc.vector.tensor_tensor(var, hT[0], hT[0], mybir.AluOpType.mult)
                nc.vector.tensor_tensor(var, qT, var, mybir.AluOpType.subtract)
                nc.vector.tensor_scalar_max(var, var, 1e-8)
                nc.scalar.sqrt(hT[1], var)

                # ---- MLP (feature-major, all 512 edges in one free dim) ----
                l1_ps = ps_mlp.tile([MLP_H, MC], f32, tag="mlp", name="l1_ps")
                for b in range(3):
                    nc.tensor.matmul(l1_ps, w1T_sb[:, b, :], hT[b],
                                     start=(b == 0), stop=(b == 2))
                l1 = mlp.tile([MLP_H, MC], f32, tag="l1s", name="l1")
                nc.scalar.activation(l1, l1_ps,
                                     mybir.ActivationFunctionType.Relu,
                                     bias=b1_sb[:, 0:1], scale=1.0)
                l2_ps = ps_mlp.tile([32, MC], f32, tag="mlp", name="l2_ps")
                nc.tensor.matmul(l2_ps, w2T_sb, l1, start=True, stop=True)
                l2 = mlp.tile([32, MC], f32, tag="l2s", name="l2")
                nc.scalar.activation(l2, l2_ps,
                                     mybir.ActivationFunctionType.Relu,
                                     bias=b2_sb[:, 0:1], scale=1.0)
                l3_ps = ps_mlp.tile([1, MC], f32, tag="mlp", name="l3_ps")
                nc.tensor.matmul(l3_ps, w3T_sb, l2, start=True, stop=True)
                w_sb = mlp.tile([1, MC], f32, tag="w", name="w_sb")
                nc.scalar.activation(w_sb, l3_ps,
                                     mybir.ActivationFunctionType.Sigmoid,
                                     bias=b3_sb[:, 0:1], scale=1.0)
                nc.sync.dma_start(out_d[:, :], w_sb)

    nc.finalize()
    return nc


def kernel(**inputs):
    from concourse import bass2jax

    in_maps, eids, kg = _prep(**inputs)
    nc = _build(kg)
    res = bass2jax.run_bass_via_pjrt(nc, in_maps, n_cores=C)
    out = np.empty(M, np.float32)
    for c in range(C):
        out[eids[c]] = res[c]["out"].reshape(MC)
    return out

